# revision 21
# baseline (speedup 1.0000x reference)
"""Trainium2 Bass kernel for nn_Attention_3934190044008.

Multi-head attention with additive bias and sigmoid gating:
  q = (q_x @ w_q), k = kv_x @ w_k, v = kv_x @ w_v   (8 heads x 64)
  a = softmax(q k^T / 8 + bias);  o = a @ v
  o = o * sigmoid(q_x @ w_g + b_g);  out = o @ w_o + b_o

Sharding: 16 (batch, head) pairs over 8 cores -> each core owns one batch
element and 2 heads, produces per-head unnormalized partials (transposed,
(o*g)^T @ ... laid out [c, q]) plus softmax denominators; the host
divides, transposes, sums and adds b_o.

Device-side layout is feature-on-partitions (transposed): scores are
computed as S^T [k, q].  The additive bias enters as a HOST-precomputed
exp(bias) factor (bf16): E = exp(S/8 - ln16) * EB, so the device never
adds the bias tensor -- the scalar engine does one exp pass (PSUM->SBUF)
and the vector engine one 2x-mode bf16 multiply pass.  A ones column in
V accumulates the softmax denominator during the AV matmul.

All matmuls are bf16 (fp8 modes are 2x faster but quantization noise in
the attention-value path does not average out -- measured ~3-5e-2 vs the
2e-2 gate; bf16 lands ~5e-3).
"""

import os
import sys
import math
import threading
from contextlib import ExitStack

import numpy as np
import ml_dtypes

_REPO = "/opt/trn_rl_repo"
if _REPO not in sys.path and os.path.isdir(_REPO):
    sys.path.insert(0, _REPO)

import concourse.bass as bass  # noqa: E402
import concourse.mybir as mybir  # noqa: E402
import concourse.tile as tile  # noqa: E402
from concourse import bacc  # noqa: E402
from concourse.bass_utils import run_bass_kernel_spmd  # noqa: E402

F32 = mybir.dt.float32
BF16 = mybir.dt.bfloat16

NP_BF16 = ml_dtypes.bfloat16

B, SEQ, CQ = 2, 2048, 256
H, DH = 8, 64
N_CORES = 8
HPC = 2          # heads per core
P = 128
NKT = SEQ // P   # 16 k-tiles

LN16 = math.log(16.0)


def build_nc():
    nc = bacc.Bacc("TRN2", target_bir_lowering=False, debug=False)

    qxT = nc.dram_tensor("qxT", [P, 2, SEQ], BF16, kind="ExternalInput").ap()
    kvxT = nc.dram_tensor("kvxT", [P, 2, SEQ], BF16, kind="ExternalInput").ap()
    ebT = nc.dram_tensor("ebT", [HPC, NKT, P, SEQ], BF16, kind="ExternalInput").ap()
    wq = nc.dram_tensor("wq", [P, 2, P], BF16, kind="ExternalInput").ap()
    wk = nc.dram_tensor("wk", [P, 2, P], BF16, kind="ExternalInput").ap()
    wv = nc.dram_tensor("wv", [P, 2, P], BF16, kind="ExternalInput").ap()
    wg = nc.dram_tensor("wg", [P, 2, P], BF16, kind="ExternalInput").ap()
    bg = nc.dram_tensor("bg", [P, 1], F32, kind="ExternalInput").ap()
    wo = nc.dram_tensor("wo", [DH, HPC, CQ], BF16, kind="ExternalInput").ap()
    # transposed per-head unnormalized partials: [c-chunk, c%128, q]
    outs_d = [nc.dram_tensor(f"out{h}", [2, P, SEQ], F32, kind="ExternalOutput").ap()
              for h in range(HPC)]
    rs_d = nc.dram_tensor("rs", [1, HPC, SEQ], F32, kind="ExternalOutput").ap()

    with tile.TileContext(nc) as tc:
        with ExitStack() as ctx:
            singles = ctx.enter_context(tc.tile_pool(name="singles", bufs=1))

            # ---- resident SBUF tensors ----
            # kv-path first on both queues: it gates the first projection
            qxT_sb = singles.tile([P, 2, SEQ], BF16)
            kvxT_sb = singles.tile([P, 2, SEQ], BF16)
            for a in range(2):
                (nc.sync if a == 0 else nc.gpsimd).dma_start(
                    kvxT_sb[:, a, :], kvxT[:, a, :])
            w_sbs = {}
            for name, src in (("wk", wk), ("wv", wv), ("wq", wq), ("wg", wg)):
                t = singles.tile([P, 2, P], BF16, tag=f"w_{name}")
                eng = nc.sync if name in ("wk", "wq") else nc.gpsimd
                eng.dma_start(t, src)
                w_sbs[name] = t
            for a in range(2):
                (nc.sync if a == 0 else nc.gpsimd).dma_start(
                    qxT_sb[:, a, :], qxT[:, a, :])
            bg_sb = singles.tile([P, 1], F32)
            nc.sync.dma_start(bg_sb, bg)
            expb_sb = singles.tile([P, 1], F32)  # -ln16 bias for the exp
            nc.vector.memset(expb_sb, -LN16)
            wo_sb = singles.tile([DH, HPC, CQ], BF16)
            nc.gpsimd.dma_start(wo_sb, wo)

            KT_sb = singles.tile([P, SEQ], BF16, name="KT")  # [2h*64, k]
            QT_sb = singles.tile([P, SEQ], BF16, name="QT")  # [2h*64, q]
            V_sb = singles.tile([P, HPC, NKT, DH + 1], BF16, name="V")
            nc.vector.memset(V_sb[:, :, :, DH:DH + 1], 1.0)
            G_sb = singles.tile([P, SEQ], BF16, name="G")    # [2h*64, q]
            OG_sb = singles.tile([DH, HPC, SEQ], BF16, name="OG")  # (o*g)^T
            rs_sb = singles.tile([1, HPC, SEQ], F32, name="rs")

            # ---- stage B: projections in a clean prologue ----
            # All PE stationary/weight churn happens here so the attention
            # loop keeps a perfectly regular PE stream (the HAM activity
            # governor only grants full PE speed after sustained gap-free
            # windows; any bubble re-throttles to 50% duty for ~3.4us+).
            with tc.tile_pool(name="ppsum", bufs=4, space="PSUM") as ppool:
                def proj_128(wt, x_sb, tt):
                    ps = ppool.tile([P, 512], F32, tag="proj")
                    nc.tensor.matmul(ps, wt[:, 0, :],
                                     x_sb[:, 0, bass.ts(tt, 512)],
                                     start=True, stop=False)
                    nc.tensor.matmul(ps, wt[:, 1, :],
                                     x_sb[:, 1, bass.ts(tt, 512)],
                                     start=False, stop=True)
                    return ps

                # K then Q feed the first score matmul; G's sigmoid runs
                # before the first exp (one ACT table swap total); V^T is
                # transposed to [k, hd] by 16 small XBAR dmas.
                for tt in range(SEQ // 512):
                    ps = proj_128(w_sbs["wk"], kvxT_sb, tt)
                    nc.scalar.copy(KT_sb[:, bass.ts(tt, 512)], ps)
                for tt in range(SEQ // 512):
                    ps = proj_128(w_sbs["wq"], qxT_sb, tt)
                    nc.vector.tensor_copy(QT_sb[:, bass.ts(tt, 512)], ps)
                for tt in range(SEQ // 512):
                    ps = proj_128(w_sbs["wg"], qxT_sb, tt)
                    nc.scalar.activation(
                        G_sb[:, bass.ts(tt, 512)], ps,
                        mybir.ActivationFunctionType.Sigmoid, bias=bg_sb)
                # V: x-stationary so out rows = tokens(k); both heads per op
                for kt in range(NKT):
                    ps = ppool.tile([P, P], F32, tag="proj", name="vproj")
                    nc.tensor.matmul(ps, kvxT_sb[:, 0, bass.ts(kt, P)],
                                     w_sbs["wv"][:, 0, :],
                                     start=True, stop=False)
                    nc.tensor.matmul(ps, kvxT_sb[:, 1, bass.ts(kt, P)],
                                     w_sbs["wv"][:, 1, :],
                                     start=False, stop=True)
                    nc.vector.tensor_copy(
                        V_sb[:, :, kt, 0:DH],
                        ps.rearrange("p (h d) -> p h d", h=HPC))

            # ---- stage C: attention ----
            with tc.tile_pool(name="spsum", bufs=2, space="PSUM") as spool, \
                 tc.tile_pool(name="otpsum", bufs=1, space="PSUM") as otpool, \
                 tc.tile_pool(name="ebp", bufs=6) as ebpool, \
                 tc.tile_pool(name="esp", bufs=3) as espool, \
                 tc.tile_pool(name="epp", bufs=3) as eppool, \
                 tc.tile_pool(name="osb", bufs=4) as opool:

                def emit_outproj(h, cc):
                    # out^T[c, q] = wo_h[hd, c].T @ OG_h[hd, q]; one
                    # stationary load per 4 matmuls
                    for qt in range(SEQ // 512):
                        ps = spool.tile([P, 512], F32, tag="s", name="fin")
                        nc.tensor.matmul(
                            ps, wo_sb[:, h, bass.ts(cc, P)],
                            OG_sb[:, h, bass.ts(qt, 512)],
                            start=True, stop=True)
                        o_sb = opool.tile([P, 512], F32, tag="osb")
                        if qt % 2 == 0:
                            nc.scalar.copy(o_sb, ps)
                        else:
                            nc.vector.tensor_copy(o_sb, ps)
                        dma = nc.sync if qt % 2 == 0 else nc.gpsimd
                        dma.dma_start(outs_d[h][cc, :, bass.ts(qt, 512)], o_sb)

                for h in range(HPC):
                    hsl = slice(h * DH, (h + 1) * DH)
                    OT = otpool.tile([DH + 1, SEQ], F32, name=f"OT{h}", tag="ot")
                    for kt in range(NKT):
                        # head 0's output projection rides head 1's loop
                        if h == 1 and kt in (4, 8):
                            emit_outproj(0, kt // 4 - 1)
                        eb = ebpool.tile([P, SEQ], BF16, tag="eb")
                        (nc.sync if kt % 2 == 0 else nc.gpsimd).dma_start(
                            eb, ebT[h, kt])
                        Es = espool.tile([P, SEQ], BF16, tag="es")
                        for half in range(2):
                            S = spool.tile([P, SEQ // 2], F32, tag="s")
                            for j in range(2):
                                nc.tensor.matmul(
                                    S[:, bass.ts(j, 512)],
                                    KT_sb[hsl, bass.ts(kt, P)],
                                    QT_sb[hsl, bass.ds(half * 1024 + j * 512, 512)],
                                    start=True, stop=True)
                            # E_s = exp(S/8 - ln16): 1/8 is the attention
                            # scale, /16 cancels in the final normalization
                            nc.scalar.activation(
                                Es[:, bass.ds(half * 1024, 1024)], S,
                                mybir.ActivationFunctionType.Exp,
                                bias=expb_sb, scale=0.125)
                        # E' = E_s * exp(bias), bf16 (DVE 2x mode)
                        Ep = eppool.tile([P, SEQ], BF16, tag="ep")
                        nc.vector.tensor_mul(Ep, Es, eb)
                        for j in range(4):
                            nc.tensor.matmul(
                                OT[:, bass.ts(j, 512)],
                                V_sb[:, h, kt, :],
                                Ep[:, bass.ts(j, 512)],
                                start=(kt == 0), stop=(kt == NKT - 1))
                    # epilogue: gate multiply + denominator export
                    nc.vector.tensor_mul(OG_sb[:, h, :], OT[0:DH, :], G_sb[hsl, :])
                    nc.scalar.copy(rs_sb[:, h, 0:1024], OT[DH:DH + 1, 0:1024])
                    nc.vector.tensor_copy(rs_sb[:, h, 1024:2048],
                                          OT[DH:DH + 1, 1024:2048])

                for h, cc in ((1, 0), (1, 1)):
                    emit_outproj(h, cc)

            nc.sync.dma_start(rs_d, rs_sb)

    nc.compile()
    return nc


_NC = None
_NC_LOCK = threading.Lock()


def _get_nc():
    global _NC
    with _NC_LOCK:
        if _NC is None:
            _NC = build_nc()
        return _NC


def make_in_maps(q_x, kv_x, bias, w_q, w_k, w_v, w_g, b_g, w_o, b_o):
    del b_o  # added on the host after the gather
    q_x = np.asarray(q_x, dtype=np.float32)
    kv_x = np.asarray(kv_x, dtype=np.float32)
    bias = np.asarray(bias, dtype=np.float32)
    w_q = np.asarray(w_q, dtype=np.float32)
    w_k = np.asarray(w_k, dtype=np.float32)
    w_v = np.asarray(w_v, dtype=np.float32)
    w_g = np.asarray(w_g, dtype=np.float32)
    b_g = np.asarray(b_g, dtype=np.float32)
    w_o = np.asarray(w_o, dtype=np.float32)

    def xt(x):  # [seq, 256] -> [128, 2, seq]
        return np.ascontiguousarray(
            x.T.reshape(2, P, SEQ).transpose(1, 0, 2)).astype(NP_BF16)

    def wt(w, cols):  # [256, hd] -> [128, 2, 128]
        return np.ascontiguousarray(
            w[:, cols].reshape(2, P, P).transpose(1, 0, 2)).astype(NP_BF16)

    in_maps = []
    for c in range(N_CORES):
        b = c // (N_CORES // B)
        h0 = HPC * (c % (N_CORES // B))
        cols = slice(h0 * DH, (h0 + HPC) * DH)
        # exp(bias)^T: [h, q, k] -> [h, kt, k%128, q]
        eb = np.exp(bias[b, h0:h0 + HPC])           # [2, q, k]
        ebT = eb.transpose(0, 2, 1).reshape(HPC, NKT, P, SEQ)
        in_maps.append({
            "qxT": xt(q_x[b]),
            "kvxT": xt(kv_x[b]),
            "ebT": np.ascontiguousarray(ebT).astype(NP_BF16),
            "wq": wt(w_q, cols),
            "wk": wt(w_k, cols),
            "wv": wt(w_v, cols),
            "wg": wt(w_g, cols),
            "bg": np.ascontiguousarray(b_g[cols].reshape(P, 1)),
            "wo": np.ascontiguousarray(
                w_o[cols, :].reshape(HPC, DH, CQ).transpose(1, 0, 2)).astype(NP_BF16),
        })
    return in_maps


def gather_output(results, b_o):
    full = np.zeros((B, SEQ, CQ), dtype=np.float32)
    for c in range(N_CORES):
        b = c // (N_CORES // B)
        rs = np.asarray(results[c]["rs"], dtype=np.float32)[0]
        for h in range(HPC):
            # out{h}: [2, 128, q] transposed partial -> [q, 256]
            ot = np.asarray(results[c][f"out{h}"], dtype=np.float32)
            full[b] += ot.reshape(CQ, SEQ).T / rs[h][:, None]
    full += np.asarray(b_o, dtype=np.float32)
    return full


def kernel(**inputs):
    nc = _get_nc()
    in_maps = make_in_maps(**inputs)
    res = run_bass_kernel_spmd(nc, in_maps, core_ids=list(range(N_CORES)))
    return gather_output(res.results, inputs["b_o"])
